# revision 31
# baseline (speedup 1.0000x reference)
"""Trainium2 Bass kernel for DGLFeatureGAT (dense GATv2 over complete graph).

Reference (per batch b, head h; N=64 nodes, D=128 feat dim):
    el = xn @ Wl,  er = xn @ Wr                      # [N, D]
    e[i,j] = sum_d a[h,d] * lrelu(el[j,d] + er[i,d])
    alpha = softmax_j(e);  rst[i,d] = sum_j alpha[i,j] el[j,d] + bias
    out = mean_h(rst) transposed to [D, N]

Decomposition (slope s=0.2):
    lrelu(z) = 0.8*relu(z) + s*z; with z = el_j + er_i the s*z part splits
    into s*u_j (folded into g_j = exp(s*u_j), multiplied into the
    aggregation rhs) and s*v_i (softmax-invariant, dropped).

    Key identity: relu(el_j + er_i) = er_i + max(el_j, -er_i), and
    sum_d 0.8*a_d*er_id is constant over j -> softmax-invariant. So a
    unit's logits may be computed EITHER as
        E_ij   = sum_d 0.8 a_d relu(el_jd + er_id)          (relu form)
    or  E'_ij  = sum_d 0.8 a_d max(el_jd, -er_id)           (max form)
    as long as a single unit (one softmax) uses one form throughout.

Pair-space layout per (b, h) unit (4096 pairs = 8 chunks x 512):
    pair p = 512c + 64*lam + i; node j sits at slot r = 32*jl0 + 4c + jhi
    (lam = 2*jhi + jl0, r = natural node id).  zabs[d, p] (partition=d).
    e-reduce: 4 matmuls/chunk (stationary zabs [128,128] slice, moving
    0.8*a_h column) land e DENSE in PSUM R2 [128=64*jl0+i, 32=4c+jhi].

Routes (per unit, homogeneous form):
    A (relu form): PE z-matmul (bf16 [el;er] stationary, fp8 selector
      moving, 512 cols) + ACT Relu PSUM->SBUF.
    D/P (max form): one DVE/Pool scalar_tensor_tensor per chunk:
      out[128,512] = max(erTn tiled x8 (stride-0 view), elT expanded x64),
      erTn = -er^T.  No PSUM, no PE.

Post per unit: exp (ACT, [128,32]) -> PE transpose -> pTu copy -> agg as
2 accumulating matmuls (pTu col-halves x elg row-halves; no gather DMAs)
-> recip + t_h scale -> PE transpose accumulate over heads -> ACT
bias+0.5 -> one [128, 256] y DMA at the end (host re-transposes).

Sharding: pure data-parallel, B=32 -> 4 batches per core x 8 cores.
"""

import numpy as np
from contextlib import ExitStack

import concourse.bass as bass
import concourse.bacc as bacc
import concourse.tile as tile
from concourse import mybir
from concourse.bass_utils import run_bass_kernel_spmd

f32 = mybir.dt.float32
bf16 = mybir.dt.bfloat16
fp8 = mybir.dt.float8e4
Act = mybir.ActivationFunctionType
Alu = mybir.AluOpType

B, W, F, H, D = 32, 128, 64, 2, 128
NEG_SLOPE = 0.2
N_CORES = 8
B_LOC = B // N_CORES            # 4 batches per core
N = F                           # 64 nodes
NCHUNK = 8
NU = B_LOC * H                  # 8 units per core

# blob column layout (f32 cols; bf16 regions packed 2-per-col).
OFF_X = 0                            # [128, 128]: x bf16 [w, 4*64 n]
OFF_XT = OFF_X + B_LOC * N // 2      # [128, 128]: x bf16 slot-order tau
OFF_WLR = OFF_XT + B_LOC * N // 2    # [128, 257]: Wl | Wr | wl_u bf16
OFF_ACOL = OFF_WLR + 257             # [128, 1]: 0.8*a bf16 pair (h0, h1)
OFF_BIAS = OFF_ACOL + 1              # [128, 1]: fused output bias f32
NCOLS_1 = OFF_BIAS + 1               # first DMA: everything above
OFF_IDB = NCOLS_1                    # [128, 128]: identity f32
NCOLS = OFF_IDB + 128
NCOLS_S = N * N // 4                 # selector fp8-packed

A_UNITS = (1, 3, 5)                  # relu-form units (h1 of b0, b1, b2)
D_ORDER = [0, 2, 4, 6, 7]            # max-form unit processing order
# chunks routed to Pool within max-form units
POOL_CHUNKS = {(u, 1) for u in D_ORDER} | {(4, 3), (6, 3)}
A_PAIRS = {(0, 3), (2, 3)}           # extra relu-route pairs (h0 units)
A_LEAD = 3                           # D-only slots before A-chunks start

_cache = {}


def _mk_slots():
    """Global emission schedule. Max-form units emit chunk-PAIRS
    (u, cp, 2); A units emit single chunks (u, c, 1).  Bresenham-merge
    the two streams, holding A back while the selector DMA lands."""
    dslots = [(u, cp, 2) for u in D_ORDER for cp in range(NCHUNK // 2)]
    aslots = [(u, c, 1) for u in A_UNITS for c in range(NCHUNK)]
    if not aslots:
        return dslots
    out = dslots[:A_LEAD]
    rest = dslots[A_LEAD:]
    rate = len(aslots) / max(1, len(rest) - 4)
    err = 0.0
    ai = 0
    for d in rest:
        err += rate
        while err >= 1.0 and ai < len(aslots):
            out.append(aslots[ai])
            ai += 1
            err -= 1.0
        out.append(d)
    out.extend(aslots[ai:])
    return out


def _build():
    if "nc" in _cache:
        return _cache["nc"]
    nc = bacc.Bacc("TRN2", target_bir_lowering=False, debug=False)
    blob_d = nc.declare_dram_parameter("blob", [128, NCOLS], f32,
                                       isOutput=False).ap()
    blobs_d = nc.declare_dram_parameter("blobS", [128, NCOLS_S], f32,
                                        isOutput=False).ap()
    y_d = nc.declare_dram_parameter("y", [D, B_LOC * N], f32,
                                    isOutput=True).ap()

    with tile.TileContext(nc) as tc, ExitStack() as ctx:
        sb1 = ctx.enter_context(tc.tile_pool(name="sb1", bufs=1))
        sbE = ctx.enter_context(tc.tile_pool(name="sbE", bufs=B_LOC))
        sbT = ctx.enter_context(tc.tile_pool(name="sbT", bufs=6))
        sbZ = ctx.enter_context(tc.tile_pool(name="sbZ", bufs=5))
        sbU = ctx.enter_context(tc.tile_pool(name="sbU", bufs=2 * B_LOC))
        psP = ctx.enter_context(tc.tile_pool(name="psP", bufs=3, space="PSUM"))
        psZ = ctx.enter_context(tc.tile_pool(name="psZ", bufs=2, space="PSUM"))
        psT = ctx.enter_context(tc.tile_pool(name="psT", bufs=2, space="PSUM"))
        psR = ctx.enter_context(tc.tile_pool(name="psR", bufs=1, space="PSUM"))

        blob = sb1.tile([128, NCOLS], f32, tag="blob")
        nc.sync.dma_start(blob[:, 0:NCOLS_1], blob_d[:, 0:NCOLS_1])
        nc.sync.dma_start(blob[:, NCOLS_1:NCOLS], blob_d[:, NCOLS_1:NCOLS])
        blobS = sb1.tile([128, NCOLS_S], f32, tag="blobS")
        nc.sync.dma_start(blobS[:], blobs_d)

        def bl(off, w):
            return blob[:, off:off + w]

        xall = bl(OFF_X, B_LOC * N // 2).bitcast(bf16)       # [128, 256]
        xtau = bl(OFF_XT, B_LOC * N // 2).bitcast(bf16)      # [128, 256]
        wlr = bl(OFF_WLR, 257).bitcast(bf16)                 # [128, 514]
        ssel = blobS[:].bitcast(fp8)                         # [128, 4096]
        acol = bl(OFF_ACOL, 1).bitcast(bf16)                 # [128, 2]
        bias_ap = bl(OFF_BIAS, 1)
        identf = bl(OFF_IDB, 128)                            # [128, 128]

        y_all = sb1.tile([D, B_LOC * N], f32, tag="yall")
        Rbank = psR.tile([128, 256], f32, tag="r2")  # unit u: cols 32u+4c+m

        a_batches = sorted({u // H for u in A_UNITS})
        tp_pairs = [(b, h) for b in range(B_LOC) for h in range(H)
                    if (2 * b + h) not in A_UNITS]

        # ---- prep: projections, g, elg, transposed el/er ----
        # boot: just enough for units 0 (b0h0), 1 (b0h1 A), 2 (b1h0);
        # the rest drains one item per z-slot via side_q.
        elers, gs, elgs, bc = {}, {}, {}, {}

        def emit_prT(b, h):
            xb = xall[:, b * N:(b + 1) * N]
            xbt = xtau[:, b * N:(b + 1) * N]
            prT = psT.tile([128, 128], f32, tag="sm", name=f"prT{b}{h}")
            nc.tensor.matmul(prT[:, 0:N], wlr[:, h * D:(h + 1) * D],
                             xbt, start=True, stop=True)
            nc.tensor.matmul(prT[:, N:2 * N],
                             wlr[:, H * D + h * D:H * D + (h + 1) * D],
                             xb, start=True, stop=True)
            elT32 = sbT.tile([128, N], f32, tag="elT", name=f"elT{b}{h}")
            erT = sbT.tile([128, N], bf16, tag="erTn", name=f"erTn{b}{h}")
            nc.vector.tensor_copy(elT32[:], prT[:, 0:N])
            nc.vector.tensor_copy(erT[:], prT[:, N:2 * N])
            bc[(b, h)] = (erT, elT32)

        def emit_proj(bp):
            b0 = 2 * bp
            xpair = xall[:, b0 * N:(b0 + 2) * N]             # [128, 128]
            proj = psP.tile([128, 512], f32, tag="sm", name=f"proj{bp}")
            nc.tensor.matmul(proj[:], xpair, wlr[:, 0:512],
                             start=True, stop=True)
            proju = psP.tile([128, 2], f32, tag="sm", name=f"proju{bp}")
            nc.tensor.matmul(proju[:], xpair, wlr[:, 512:514],
                             start=True, stop=True)
            g_p = sbU.tile([128, H], f32, tag="g", name=f"g{bp}")
            nc.scalar.activation(g_p[:], proju[:], Act.Exp)
            gs[b0] = g_p[0:N, :]
            g_lo = sbU.tile([N, H], f32, tag="glo", name=f"glo{bp}")
            nc.vector.tensor_copy(g_lo[:], g_p[N:128, :])
            gs[b0 + 1] = g_lo[:]
            return proj

        def emit_el(proj, b):
            bo = b % 2
            eler = sbE.tile([128, H * D], bf16, tag="eler", name=f"eler{b}")
            nc.scalar.activation(eler[0:N, :],
                                 proj[bo * N:(bo + 1) * N, 0:H * D],
                                 Act.Identity)
            elers[b] = eler

        def emit_er(proj, b):
            bo = b % 2
            nc.vector.tensor_copy(
                elers[b][N:128, :],
                proj[bo * N:(bo + 1) * N, H * D:2 * H * D])

        def emit_elg(b, h):
            eler = elers[b]
            elg = sbU.tile([N, D + 1], bf16, tag=f"elg{b}{h}",
                           name=f"elg{b}{h}", bufs=1)
            nc.vector.tensor_scalar(
                elg[:, 0:D], eler[0:N, h * D:(h + 1) * D],
                gs[b][:, h:h + 1], None, Alu.mult)
            nc.vector.tensor_copy(elg[:, D:D + 1], gs[b][:, h:h + 1])
            elgs[(b, h)] = elg

        proj0 = emit_proj(0)
        emit_el(proj0, 0)
        emit_er(proj0, 0)           # b0 is an A batch (unit 1)
        emit_prT(0, 0)
        emit_el(proj0, 1)
        emit_prT(1, 0)
        emit_er(proj0, 1)           # b1 A batch (unit 3)

        side_q = [
            lambda: emit_elg(0, 0), lambda: emit_elg(0, 1),
            lambda: emit_elg(1, 0), lambda: emit_elg(1, 1),
        ]
        side_q.insert(0, lambda: boot2())

        def boot2():
            proj1 = emit_proj(1)
            side_q.extend([
                lambda: emit_el(proj1, 2),
                lambda: emit_er(proj1, 2),             # b2 A batch (unit 5)
                lambda: emit_prT(2, 0),
                lambda: emit_el(proj1, 3),
                lambda: emit_prT(3, 0),
                lambda: emit_prT(3, 1),
                lambda: emit_elg(2, 0), lambda: emit_elg(2, 1),
                lambda: emit_elg(3, 0), lambda: emit_elg(3, 1),
            ])


        # ---- z + e-reduce per global slot schedule ----
        zabss = {}

        def emit_z(u, c):
            b, h = u // H, u % H
            if u not in zabss:
                zabss[u] = sbZ.tile([128, N * N], bf16, tag="zabs",
                                    name=f"zabs{u}")
            zabs = zabss[u]
            if u in A_UNITS or (u, c // 2) in A_PAIRS:
                for cc in ([c] if u in A_UNITS else [c, c + 1]):
                    zc = psZ.tile([128, 512], f32, tag="zc")
                    nc.tensor.matmul(zc[:], elers[b][:, h * D:(h + 1) * D],
                                     ssel[:, 512 * cc:512 * (cc + 1)],
                                     start=True, stop=True)
                    nc.scalar.activation(zabs[:, 512 * cc:512 * (cc + 1)],
                                         zc[:], Act.Relu)
            else:
                cp = c // 2          # chunk-pair index
                erT, elT32 = bc[(b, h)]
                eng = (nc.gpsimd if (u, cp) in POOL_CHUNKS else nc.vector)
                for cc in (2 * cp, 2 * cp + 1):
                    for lam in range(NCHUNK):
                        eng.tensor_scalar(
                            zabs[:, 512 * cc + N * lam:
                                 512 * cc + N * (lam + 1)],
                            erT[:], elT32[:, NCHUNK * cc + lam:
                                          NCHUNK * cc + lam + 1],
                            0.0, Alu.add, Alu.max)

        def emit_ereduce(u, c, nch=1):
            h = u % H
            zabs = zabss[u]
            R2 = Rbank[:, 32 * u:32 * u + 32]
            for m in range(4 * nch):
                nc.tensor.matmul(
                    R2[:, 4 * c + m:4 * c + m + 1],
                    zabs[:, 512 * c + 128 * m:512 * c + 128 * (m + 1)],
                    acol[:, h:h + 1],
                    start=True, stop=True, skip_group_check=True)

        oTs = {}
        oT_seen = {}
        post_st = {}

        def post_a(u):
            stagedE = sbU.tile([128, 32], f32, tag="stE",
                               name=f"stE{u}", bufs=4)
            nc.scalar.activation(stagedE[:], Rbank[:, 32 * u:32 * u + 32],
                                 Act.Exp)
            pT_ps = psT.tile([32, 128], f32, tag="sm", name=f"pTp{u}")
            nc.tensor.matmul(pT_ps[:], stagedE[:], identf,
                             is_transpose=True)
            post_st[u] = (pT_ps,)

        def post_b(u):
            b, h = u // H, u % H
            (pT_ps,) = post_st[u]
            pTu = sbU.tile([N, N], bf16, tag="pTu",
                           name=f"pTu{u}", bufs=4)
            for jl0 in range(2):
                nc.vector.tensor_copy(pTu[32 * jl0:32 * (jl0 + 1), :],
                                      pT_ps[:, N * jl0:N * (jl0 + 1)])
            ag = psT.tile([N, D + 1], f32, tag="sm", name=f"ag{u}")
            nc.tensor.matmul(ag[:], pTu[:], elgs[(b, h)][:],
                             start=True, stop=True)
            post_st[u] = (ag,)

        def post_c(u):
            b, h = u // H, u % H
            (ag,) = post_st[u]
            r_u = sbU.tile([N, 1], f32, tag="r", name=f"r{u}", bufs=4)
            nc.vector.reciprocal(r_u[:], ag[:, D:D + 1])
            t_h = sbU.tile([N, D], f32, tag="th", name=f"th{u}", bufs=4)
            nc.vector.tensor_scalar(t_h[:], ag[:, 0:D], r_u[:], None,
                                    Alu.mult)
            if b not in oTs:
                oTs[b] = psP.tile([D, N], f32, tag="sm", name=f"oT{b}")
                oT_seen[b] = 0
            first = oT_seen[b] == 0
            oT_seen[b] += 1
            nc.tensor.matmul(oTs[b][:], t_h[:], identf[0:N, 0:N],
                             is_transpose=True,
                             start=first, stop=not first,
                             skip_group_check=True)
            if not first:
                nc.scalar.activation(y_all[:, N * b:N * (b + 1)], oTs[b][:],
                                     Act.Identity, bias=bias_ap, scale=0.5)
                nc.sync.dma_start(y_d[:, N * b:N * (b + 1)],
                                  y_all[:, N * b:N * (b + 1)])

        done = {u: 0 for u in range(NU)}
        slots = _mk_slots()
        pend = []
        post_q = []

        def run_due(i):
            while post_q and post_q[0][0] <= i:
                _, fn, uu = post_q.pop(0)
                fn(uu)

        for i, (u, c, nch) in enumerate(slots):
            run_due(i)
            emit_z(u, c if nch == 1 else 2 * c)
            if side_q:
                side_q.pop(0)()
            pend.append((u, c if nch == 1 else 2 * c, nch))
            if len(pend) > 1:
                up, cp, nc_ = pend.pop(0)
                emit_ereduce(up, cp, nc_)
                done[up] += nc_
                if done[up] == NCHUNK:
                    post_q += [(i + 1, post_a, up), (i + 3, post_b, up),
                               (i + 5, post_c, up)]
                    post_q.sort(key=lambda t: t[0])
        while side_q:
            side_q.pop(0)()
        tail_units = []
        for (up, cp, nc_) in pend:
            emit_ereduce(up, cp, nc_)
            done[up] += nc_
            if done[up] == NCHUNK:
                tail_units.append(up)
        # interleave leftover post stages across pending units
        leftover = {}
        for (due, fn, uu) in post_q:
            leftover.setdefault(uu, []).append(fn)
        post_q.clear()
        for uu in tail_units:
            leftover[uu] = [post_a, post_b, post_c]
        prog = {uu: 3 - len(fns) for uu, fns in leftover.items()}
        for stage in range(3):
            for uu, fns in leftover.items():
                idx = stage - prog[uu]
                if 0 <= idx < len(fns):
                    fns[idx](uu)

    nc.compile()
    _cache["nc"] = nc
    return nc


def _pack_bf16(a):
    """[P, 2k] f32 -> [P, k] f32 bit-packed bf16 pairs (little-endian)."""
    import ml_dtypes
    ab = a.astype(ml_dtypes.bfloat16).view(np.uint16)
    return (ab[:, 0::2].astype(np.uint32)
            | (ab[:, 1::2].astype(np.uint32) << 16)).view(np.float32)


def _pack_fp8(a):
    """[P, 4k] f32 -> [P, k] f32 bit-packed fp8e4m3 quads."""
    import ml_dtypes
    ab = a.astype(ml_dtypes.float8_e4m3fn).view(np.uint8)
    return (ab[:, 0::4].astype(np.uint32)
            | (ab[:, 1::4].astype(np.uint32) << 8)
            | (ab[:, 2::4].astype(np.uint32) << 16)
            | (ab[:, 3::4].astype(np.uint32) << 24)).view(np.float32)


def _slot_of_r(r):
    """Node r -> pair slot (chunk c, lane lam)."""
    jl0, c, jhi = r >> 5, (r >> 2) & 7, r & 3
    return c, 2 * jhi + jl0


def _make_blobs(x, Wl, Wr, attn_a, bias):
    """Host-side prep: per-core input blobs [128, NCOLS] float32."""
    x = np.asarray(x, np.float32)
    Wl = np.asarray(Wl, np.float32)
    Wr = np.asarray(Wr, np.float32)
    attn_a = np.asarray(attn_a, np.float32)
    bias = np.asarray(bias, np.float32)

    wl_u = np.einsum("whd,hd->wh", Wl.reshape(W, H, D), attn_a) * NEG_SLOPE
    wlr = np.concatenate([Wl, Wr, wl_u], axis=1)              # [128, 514]

    s_sel = np.zeros((128, N * N), np.float32)
    for r in range(N):
        c, lam = _slot_of_r(r)
        s_sel[r, 512 * c + N * lam:512 * c + N * (lam + 1)] = 1.0
    loc = np.arange(N * N)
    s_sel[N + loc % N, loc] = 1.0

    tau = np.zeros(N, np.int64)
    for r in range(N):
        c, lam = _slot_of_r(r)
        tau[8 * c + lam] = r

    a2 = (1.0 - NEG_SLOPE) * attn_a                           # [H, 128]
    a_col = np.stack([a2[0], a2[1]], axis=1)                  # [128, 2]
    bias_f = 0.5 * (bias.reshape(H, D)[0] + bias.reshape(H, D)[1])
    identb = np.eye(128, dtype=np.float32)

    sel_packed = np.ascontiguousarray(_pack_fp8(s_sel))

    blobs = []
    for core in range(N_CORES):
        xs = x[core * B_LOC:(core + 1) * B_LOC]    # [4, 128, 64]
        xsec = xs.transpose(1, 0, 2).reshape(128, B_LOC * N)
        xsect = xs[:, :, tau].transpose(1, 0, 2).reshape(128, B_LOC * N)
        ba = np.ascontiguousarray(np.concatenate(
            [_pack_bf16(xsec), _pack_bf16(xsect), _pack_bf16(wlr),
             _pack_bf16(a_col), bias_f.reshape(128, 1), identb], axis=1))
        blobs.append({"blob": ba, "blobS": sel_packed})
    return blobs


def kernel(x, Wl, Wr, attn_a, bias):
    nc = _build()
    blobs = _make_blobs(x, Wl, Wr, attn_a, bias)
    in_maps = [blobs[c] for c in range(N_CORES)]
    res = run_bass_kernel_spmd(nc, in_maps, list(range(N_CORES)))
    out = np.concatenate(
        [res.results[c]["y"].reshape(D, B_LOC, N).transpose(1, 0, 2)
         for c in range(N_CORES)], axis=0)
    return np.ascontiguousarray(out.astype(np.float32))
